# revision 1
# baseline (speedup 1.0000x reference)
"""DilatedRnnStack kernel for 8 TRN2 NeuronCores (Bass/Tile, SPMD).

Tensor-parallel over the 4*S=4096 gate dim: core j owns rows
[128j, 128j+128) of every gate of all 4 layers; per wavefront cell the
cores AllGather their fp16 [128,64] slice of `whole` (one fused
collective per wave). Layer-0's input projection is batched over all
T=16 timesteps in the prologue at N=512 (added per-cell on DVE);
dH==prevH weight chunks are pre-folded (layer0 always, layers1-3 for
t<d); 'a'-gate matmuls are skipped for t<d; recurrent weights of all
layers and the u-weights of layers 2-3 are fp8e4m3 (end-to-end rel err
~4e-3), u-weights of layers 0-1 stay fp16. The adaptor is computed in
four 256-col chunks interleaved into the wave loop so its matmuls fill
PE idle during collectives. Cells are emitted deepest-layer-first
within a wave; per-cell readback DMAs keep dependency granularity fine.
"""
import sys

sys.path.insert(0, "/opt/trn_rl_repo")
import numpy as np

DILS = (1, 2, 4, 8)
T, B = 16, 64
H, S = 256, 1024
DIN, DOUT = 512, 512
NC = 8
MT = ((0, 1, 3), (0, 1, 2, 3), (0, 1, 2, 3), (0, 1, 2, 3))


def build_kernel(exchange="agw", repeat=1, percell_dma=False, dh_first=True, adaptor4=True, wave_rb=False, desc_cells=True, gbufs=4, rb_two_q=False):
    import concourse.bacc as bacc
    import concourse.tile as tile
    from concourse import mybir

    f32 = mybir.dt.float32
    mdt = mybir.dt.float16
    f8 = mybir.dt.float8e4
    AF = mybir.ActivationFunctionType

    nc = bacc.Bacc(
        "TRN2", target_bir_lowering=False, debug=False, num_devices=NC
    )

    xt_d = nc.dram_tensor("xt", [128, 4, T * B], mdt, kind="ExternalInput")
    w0u_d = nc.dram_tensor("w0u", [128, 4, 384], mdt, kind="ExternalInput")
    w0r_d = nc.dram_tensor("w0r", [128, 2, 384], f8, kind="ExternalInput")
    w1u_d = nc.dram_tensor("w1u", [128, 6, 512], mdt, kind="ExternalInput")
    w1r_d = nc.dram_tensor("w1r", [128, 4, 512], f8, kind="ExternalInput")
    w1f_d = nc.dram_tensor("w1f", [128, 2, 512], f8, kind="ExternalInput")
    w2_d = nc.dram_tensor("w2", [128, 12, 512], f8, kind="ExternalInput")
    w3_d = nc.dram_tensor("w3", [128, 12, 512], f8, kind="ExternalInput")
    bias_d = nc.dram_tensor("bias", [128, 16], f32, kind="ExternalInput")
    wat_d = nc.dram_tensor("wat", [128, 6, 64], mdt, kind="ExternalInput")
    bay_d = nc.dram_tensor("bay", [64, 1], f32, kind="ExternalInput")
    y_d = nc.dram_tensor("y", [64, T * B], f32, kind="ExternalOutput")

    with tile.TileContext(nc) as tc:
        with (
            tc.tile_pool(name="const", bufs=1) as const,
            tc.tile_pool(name="work", bufs=6) as work,
            tc.tile_pool(name="gps", bufs=gbufs, space="PSUM") as gps,
            tc.tile_pool(name="ups", bufs=2, space="PSUM") as ups,
            tc.tile_pool(name="yps", bufs=2, space="PSUM") as yps,
            tc.tile_pool(name="dram", bufs=16, space="DRAM") as dram,
        ):
            xsb = const.tile([128, 4, T * B], mdt, tag="xsb")
            nc.sync.dma_start(xsb[:], xt_d[:])
            w0usb = const.tile([128, 4, 384], mdt, tag="w0u")
            nc.sync.dma_start(w0usb[:], w0u_d[:])
            w0rsb = const.tile([128, 2, 384], f8, tag="w0r")
            nc.sync.dma_start(w0rsb[:], w0r_d[:])
            w1usb = const.tile([128, 6, 512], mdt, tag="w1u")
            nc.sync.dma_start(w1usb[:], w1u_d[:])
            w1rsb = const.tile([128, 4, 512], f8, tag="w1r")
            nc.sync.dma_start(w1rsb[:], w1r_d[:])
            w1fsb = const.tile([128, 2, 512], f8, tag="w1f")
            nc.sync.dma_start(w1fsb[:], w1f_d[:])
            w2sb = const.tile([128, 12, 512], f8, tag="w2")
            nc.sync.dma_start(w2sb[:], w2_d[:])
            w3sb = const.tile([128, 12, 512], f8, tag="w3")
            nc.sync.dma_start(w3sb[:], w3_d[:])
            bsb = const.tile([128, 16], f32, tag="bias")
            nc.sync.dma_start(bsb[:], bias_d[:])
            watsb = const.tile([128, 6, 64], mdt, tag="wat")
            nc.sync.dma_start(watsb[:], wat_d[:])
            baysb = const.tile([64, 1], f32, tag="bay")
            nc.sync.dma_start(baysb[:], bay_d[:])

            u0sb = const.tile([128, 3, T * B], f32, tag="u0sb")
            wh_all = const.tile([128, T + 3, 4, 8, B], mdt, tag="wh_all")
            c_all = const.tile([128, T, 4, B], f32, tag="c_all")
            blockBuf = const.tile([128, 6, T * B], mdt, tag="blockBuf")
            ysb = const.tile([64, T * B], f32, tag="ysb")

            def wh(l, t):
                return wh_all[:, t + l, l, :, :]

            def bias_ap(l, mi):
                return bsb[:, 4 * l + mi : 4 * l + mi + 1]

            def u_stat(l, k, mi):
                if l == 1:
                    return w1usb[:, k, 128 * mi : 128 * (mi + 1)]
                return (w2sb if l == 2 else w3sb)[
                    :, k, 128 * mi : 128 * (mi + 1)
                ]

            def rec_stat(l, i, mi):
                if l == 0:
                    return w0rsb[:, i, 128 * mi : 128 * (mi + 1)]
                if l == 1:
                    return w1rsb[:, i, 128 * mi : 128 * (mi + 1)]
                return (w2sb if l == 2 else w3sb)[
                    :, 6 + i, 128 * mi : 128 * (mi + 1)
                ]

            def fold_stat(l, i, mi):
                if l == 0:
                    return w0rsb[:, i, 128 * mi : 128 * (mi + 1)]
                if l == 1:
                    return w1fsb[:, i, 128 * mi : 128 * (mi + 1)]
                return (w2sb if l == 2 else w3sb)[
                    :, 10 + i, 128 * mi : 128 * (mi + 1)
                ]

            def emit_cell(l, t, stage):
                d = DILS[l]
                gates = MT[l]
                if t == 0:
                    act_m = [mi for mi, g in enumerate(gates) if g in (1, 3)]
                elif t < d:
                    act_m = [mi for mi, g in enumerate(gates) if g != 2]
                else:
                    act_m = list(range(len(gates)))

                # --- matmuls ---
                g_ps = None
                if not (l == 0 and t == 0):
                    g_ps = gps.tile([128, 4 * B], f32, tag="g")
                    for mi in act_m:
                        seq = []
                        # dH chunks first: gathered d waves ago, so these
                        # matmuls can issue while the latest gather is in
                        # flight.
                        if dh_first and t >= d and d > 1 and l >= 1:
                            for i in (2, 3):
                                seq.append(
                                    (
                                        rec_stat(l, i, mi),
                                        wh(l, t - d)[:, 6 + (i % 2), :],
                                    )
                                )
                        if l >= 1:
                            for k in range(6):
                                seq.append(
                                    (u_stat(l, k, mi), wh(l - 1, t)[:, k, :])
                                )
                        if t >= 1:
                            if l == 0 or t < d:
                                for i in range(2):
                                    seq.append(
                                        (
                                            fold_stat(l, i, mi),
                                            wh(l, t - 1)[:, 6 + i, :],
                                        )
                                    )
                            else:
                                rng = range(2) if dh_first else range(4)
                                for i in rng:
                                    st = t - 1 if i < 2 else t - d
                                    seq.append(
                                        (
                                            rec_stat(l, i, mi),
                                            wh(l, st)[:, 6 + (i % 2), :],
                                        )
                                    )
                        for ci, (stat, mov) in enumerate(seq):
                            nc.tensor.matmul(
                                g_ps[:, B * mi : B * (mi + 1)],
                                stat,
                                mov,
                                start=(ci == 0),
                                stop=(ci == len(seq) - 1),
                            )

                def gsrc(mi):
                    # pre-activation input AP for gate slot mi
                    if l == 0:
                        if t == 0:
                            return u0sb[:, mi, B * t : B * (t + 1)]
                        tmp = work.tile([128, B], f32, tag="gu")
                        nc.vector.tensor_add(
                            tmp[:],
                            g_ps[:, B * mi : B * (mi + 1)],
                            u0sb[:, mi, B * t : B * (t + 1)],
                        )
                        return tmp[:]
                    return g_ps[:, B * mi : B * (mi + 1)]

                def mslot(gid):
                    return gates.index(gid)

                cnew = c_all[:, t, l, :]
                if t == 0:
                    o_t = work.tile([128, B], f32, tag="o")
                    nc.scalar.activation(
                        cnew, gsrc(mslot(1)), AF.Tanh, bias=bias_ap(l, mslot(1))
                    )
                    nc.scalar.activation(
                        o_t[:], gsrc(mslot(3)), AF.Sigmoid,
                        bias=bias_ap(l, mslot(3)),
                    )
                    nc.vector.tensor_mul(stage[:, l, :], o_t[:], cnew)
                else:
                    f_t = work.tile([128, B], f32, tag="f")
                    n_t = work.tile([128, B], f32, tag="n")
                    o_t = work.tile([128, B], f32, tag="o")
                    prevC = c_all[:, t - 1, l, :]
                    wC = None
                    if t >= d and d > 1:
                        # 'a' first so its DVE chain overlaps the f/n/o ACTs
                        a_t = work.tile([128, B], f32, tag="a")
                        t1 = work.tile([128, B], f32, tag="t1")
                        nc.scalar.activation(
                            a_t[:], gsrc(mslot(2)), AF.Sigmoid,
                            bias=bias_ap(l, mslot(2)),
                        )
                        dC = c_all[:, t - d, l, :]
                        nc.vector.tensor_sub(t1[:], prevC, dC)
                        nc.vector.tensor_mul(t1[:], a_t[:], t1[:])
                        nc.vector.tensor_add(t1[:], t1[:], dC)
                        wC = t1[:]
                    nc.scalar.activation(
                        f_t[:], gsrc(mslot(0)), AF.Sigmoid,
                        bias=bias_ap(l, mslot(0)),
                    )
                    nc.scalar.activation(
                        n_t[:], gsrc(mslot(1)), AF.Tanh,
                        bias=bias_ap(l, mslot(1)),
                    )
                    nc.scalar.activation(
                        o_t[:], gsrc(mslot(3)), AF.Sigmoid,
                        bias=bias_ap(l, mslot(3)),
                    )
                    if wC is None:
                        wC = prevC
                    t2 = work.tile([128, B], f32, tag="t2")
                    nc.vector.tensor_sub(t2[:], wC, n_t[:])
                    nc.vector.tensor_mul(t2[:], f_t[:], t2[:])
                    nc.vector.tensor_add(cnew, t2[:], n_t[:])
                    nc.vector.tensor_mul(stage[:, l, :], o_t[:], cnew)

            def emit_adaptor(h, cw=256):
                yp = yps.tile([64, cw], f32, tag="yp")
                for jc in range(6):
                    nc.tensor.matmul(
                        yp[:],
                        watsb[:, jc, :],
                        blockBuf[:, jc, cw * h : cw * (h + 1)],
                        start=(jc == 0),
                        stop=(jc == 5),
                    )
                nc.scalar.activation(
                    ysb[:, cw * h : cw * (h + 1)],
                    yp[:],
                    AF.Identity,
                    bias=baysb[:, 0:1],
                )

            for rep in range(repeat):
                # --- batched layer-0 u-projection (prologue) ---
                for mi, hh in ((1, 0), (2, 0), (0, 0), (1, 1), (2, 1), (0, 1)):
                    if True:
                        up = ups.tile([128, 512], f32, tag="up")
                        for k in range(4):
                            nc.tensor.matmul(
                                up[:],
                                w0usb[:, k, 128 * mi : 128 * (mi + 1)],
                                xsb[:, k, 512 * hh : 512 * (hh + 1)],
                                start=(k == 0),
                                stop=(k == 3),
                            )
                        nc.scalar.activation(
                            u0sb[:, mi, 512 * hh : 512 * (hh + 1)],
                            up[:],
                            AF.Identity,
                        )

                # --- wavefront ---
                for w in range(T + 3):
                    cells = [(l, w - l) for l in range(4) if 0 <= w - l < T]
                    if desc_cells:
                        cells = cells[::-1]
                    lmin = min(l for l, _ in cells)
                    nv = len(cells)
                    stage = work.tile(
                        [128, 4, B], mdt, tag="wstage", name="wstage"
                    )
                    for l, t in cells:
                        emit_cell(l, t, stage)
                    b_in = dram.tile(
                        [128, nv * (B // 2)], f32, tag="bin", name="bin",
                        bufs=4,
                    )
                    b_out = dram.tile(
                        [S, nv * (B // 2)], f32, addr_space="Shared",
                        tag="bout", name="bout", bufs=4,
                    )
                    # per-cell staging: each slice ships as soon as its
                    # cell's output mul lands, instead of waiting for the
                    # whole wave
                    if percell_dma:
                        for l, t in cells:
                            li = l - lmin
                            nc.scalar.dma_start(
                                b_in[:, 32 * li : 32 * (li + 1)],
                                stage[:, l, :].bitcast(f32),
                            )
                    else:
                        nc.scalar.dma_start(
                            b_in[:],
                            stage[:, lmin : lmin + nv, :].bitcast(f32),
                        )
                    if exchange == "agw":
                        nc.gpsimd.collective_compute(
                            "AllGather",
                            mybir.AluOpType.bypass,
                            ins=[b_in[:].opt()],
                            outs=[b_out[:].opt()],
                            replica_groups=[list(range(NC))],
                        )
                    else:  # timing-only diagnostic
                        nc.sync.dma_start(b_out[0:128, :], b_in[:])
                    bo = b_out[:].rearrange(
                        "(j p) (l b) -> p l j b", p=128, l=nv
                    )
                    if wave_rb:
                        nc.sync.dma_start(
                            wh_all[:, w, lmin : lmin + nv, :, :].bitcast(f32),
                            bo[:, :, :, :],
                        )
                    else:
                        for ci, (l, t) in enumerate(cells):
                            eng = nc.scalar if (rb_two_q and ci % 2) else nc.sync
                            eng.dma_start(
                                wh(l, t).bitcast(f32),
                                bo[:, l - lmin, :, :],
                            )
                    for l, t in cells:
                        if l == 1:
                            nc.vector.tensor_copy(
                                blockBuf[:, :, B * t : B * (t + 1)],
                                wh(1, t)[:, 0:6, :],
                            )
                        elif l == 3:
                            nc.vector.tensor_add(
                                blockBuf[:, :, B * t : B * (t + 1)],
                                blockBuf[:, :, B * t : B * (t + 1)],
                                wh(3, t)[:, 0:6, :],
                            )
                    if adaptor4:
                        if w in (6, 10, 14, 18):
                            emit_adaptor((w - 6) // 4, 256)
                    else:
                        if w == 10:
                            emit_adaptor(0, 512)
                        elif w == 18:
                            emit_adaptor(1, 512)

            nc.sync.dma_start(y_d[:], ysb[:])

    nc.compile()
    return nc


def prep_inputs(inputs):
    import ml_dtypes

    f8 = ml_dtypes.float8_e4m3
    x = np.ascontiguousarray(inputs["x"], dtype=np.float32)
    Ws = [np.asarray(inputs[f"W{l}"], np.float32) for l in range(4)]
    bs = [np.asarray(inputs[f"b{l}"], np.float32) for l in range(4)]
    Wa = np.asarray(inputs["Wa"], np.float32)
    ba = np.asarray(inputs["ba"], np.float32)

    # x feature-major: xt[p, c, t*B+b] = x[t, b, 128c+p]
    xt = x.reshape(T * B, 4, 128).transpose(2, 1, 0)
    xt = np.ascontiguousarray(
        xt.transpose(0, 1, 2), dtype=np.float16
    )  # [128, 4, T*B]

    def wtile(Wsel, dtype):
        # Wsel: [nm*128 rows, K] -> [128 p, K/128, nm*128]
        nm = Wsel.shape[0] // 128
        nk = Wsel.shape[1] // 128
        wt = Wsel.T.reshape(nk, 128, nm * 128).transpose(1, 0, 2)
        return np.ascontiguousarray(wt, dtype=dtype)

    in_maps = []
    for j in range(NC):
        def rows(l):
            return np.concatenate(
                [
                    np.arange(g * S + 128 * j, g * S + 128 * (j + 1))
                    for g in MT[l]
                ]
            )

        m = {"xt": xt}
        r0 = rows(0)
        W0 = Ws[0][r0, :]
        m["w0u"] = wtile(W0[:, :512], np.float16)
        m["w0r"] = wtile(W0[:, 512:768] + W0[:, 768:1024], f8)
        r1 = rows(1)
        W1 = Ws[1][r1, :]
        m["w1u"] = wtile(W1[:, :768], np.float16)
        m["w1r"] = wtile(W1[:, 768:1280], f8)
        m["w1f"] = wtile(W1[:, 768:1024] + W1[:, 1024:1280], f8)
        for l in (2, 3):
            Wl = Ws[l][rows(l), :]
            cat = np.concatenate(
                [Wl[:, :1280], Wl[:, 768:1024] + Wl[:, 1024:1280]], axis=1
            )
            m[f"w{l}"] = wtile(cat, f8)
        bias = np.zeros((128, 16), np.float32)
        for l in range(4):
            for mi, g in enumerate(MT[l]):
                v = bs[l][g * S + 128 * j : g * S + 128 * (j + 1)].copy()
                if g == 0:
                    v += 1.0
                bias[:, 4 * l + mi] = v
        m["bias"] = bias
        wa_j = Wa[64 * j : 64 * (j + 1), :]
        wat = wa_j.T.reshape(6, 128, 64).transpose(1, 0, 2)
        m["wat"] = np.ascontiguousarray(wat, dtype=np.float16)
        m["bay"] = np.ascontiguousarray(ba[64 * j : 64 * (j + 1), None])
        in_maps.append(m)
    return in_maps


def assemble_output(res):
    y = np.zeros((T, B, DOUT), np.float32)
    for j in range(NC):
        yj = res[j]["y"].reshape(64, T, B)
        y[:, :, 64 * j : 64 * (j + 1)] = yj.transpose(1, 2, 0)
    return y


_CACHED_NC = None


def kernel(**inputs) -> np.ndarray:
    global _CACHED_NC
    from concourse import bass_utils

    if _CACHED_NC is None:
        _CACHED_NC = build_kernel()
    in_maps = prep_inputs(inputs)
    res = bass_utils.run_bass_kernel_spmd(
        _CACHED_NC, in_maps, core_ids=list(range(NC))
    )
    return assemble_output([res.results[j] for j in range(NC)])

